# revision 49
# baseline (speedup 1.0000x reference)
"""MoE router kernel for Trainium2 (8 NeuronCores, data-parallel over tokens).

Computes, for x [4, 4096, 4096] f32 and W_router [8, 4096] f32:
    scores  = x_flat @ W_router.T          # [16384, 8]
    probs   = softmax(scores)
    w, idx  = top_k(probs, 2); w /= w.sum(-1, keepdims=True)

Identity used on-device: the renormalized top-2 softmax weights depend only on
the top-2 score gap:  w1 = 1/(1+e^(s2-s1)), w2 = e^(s2-s1)/(1+e^(s2-s1)).

Sharding: tokens split 8 ways (2048/core), W_router.T replicated. Each shard is
pre-swizzled on the host to xq[n, p, k, ti] = x[n*512+ti, k*128+p] so (a) the
PE contracts over hidden (which must sit on the partition axis) with no
on-device transpose, and (b) every DMA tile reads one contiguous 64KB run per
partition — measured at the per-core HBM roofline (~86us for 32MB).

Default variant "ac": W^T chunks [128,8] stationary, x [128,512] moving (fp32,
exact), with 4 k-chunks column-packed into distinct 32-column strips of the PE
array via tile_position so their moving streams overlap — this hides the fp32
4-cycles/row stream cost under the DMA (measured ~80us/core vs 117us unpacked
and 210us for the x-stationary orientation). scores^T partials are summed and
PE-transposed back to [token, expert], then DVE Max8/MaxIndex pick the top-2
and an exp/reciprocal chain forms the weights.
"""

import os

import numpy as np

import concourse.bass as bass
import concourse.tile as tile
from concourse import bass_utils, mybir

N_CORES = 8
HIDDEN = 4096
N_EXPERTS = 8
TOKENS_TOTAL = 16384
TOKENS_PER_CORE = TOKENS_TOTAL // N_CORES  # 2048
TOKEN_TILE = 512  # tokens per DMA tile (one contiguous 64KB run per partition)
SUB = TOKEN_TILE // 128  # 128-token matmul sub-blocks per tile
N_TOKEN_TILES = TOKENS_PER_CORE // TOKEN_TILE  # 4
K_CHUNKS = HIDDEN // 128  # 32

X_BUFS = 2
K_SPLIT = 4  # sub-DMAs per token tile (each [128, K_CHUNKS/K_SPLIT, TOKEN_TILE])
DEFAULT_VARIANT = "ac"

LAST_EXEC_NS = None


def _split_multiwait(nc: bass.Bass) -> None:
    """The walrus in this container rejects >1 sync wait per instruction.
    Hoist excess waits onto single-wait NOPs inserted just before, on the
    same engine queue (semantically identical: waits execute in order)."""
    n = 0
    for f in nc.m.functions:
        for b in f.blocks:
            insts = b.instructions
            if not any(
                i.sync_info is not None and len(i.sync_info.on_wait) > 1
                for i in insts
            ):
                continue
            new = []
            for inst in insts:
                si = inst.sync_info
                if si is not None and len(si.on_wait) > 1:
                    waits = list(si.on_wait)
                    for w in waits[:-1]:
                        n += 1
                        new.append(
                            mybir.InstNoOp(
                                name=f"I-splitwait-{n}",
                                engine=inst.engine,
                                sync_info=mybir.SyncInfo(
                                    on_wait=[w], on_update=[]
                                ),
                                bass_nofuse=True,
                            )
                        )
                    inst.sync_info = mybir.SyncInfo(
                        on_wait=[waits[-1]], on_update=list(si.on_update)
                    )
                new.append(inst)
            b.instructions = new


def build_kernel(repeat: int = 1, variant: str = DEFAULT_VARIANT) -> bass.Bass:
    f32 = mybir.dt.float32
    u32 = mybir.dt.uint32

    # float32r = same bytes as f32; PE streams it at full rate (N>=256) but
    # with reduced mantissa (~7 index flips / 32k tokens — not used by default).
    xdt = mybir.dt.float32r if variant == "a" else f32
    a_like = variant in ("a", "af", "ac")

    nc = bass.Bass("TRN2", target_bir_lowering=False, debug=False)
    # xq[n, p, k, ti] = x[n*TOKEN_TILE + ti, k*128 + p] — pre-swizzled on host
    # so each DMA tile reads one contiguous (K_CHUNKS*TOKEN_TILE*4)B run per
    # partition.
    xq = nc.dram_tensor(
        "xq",
        [N_TOKEN_TILES, 128, K_CHUNKS, TOKEN_TILE],
        xdt,
        kind="ExternalInput",
    ).ap()
    # wq[p, k, e] = W_router[e, k*128+p] — host-swizzled to the SBUF layout so
    # the load is 128 x 1KB contiguous runs (vs 4096 x 32B from [hidden, 8]).
    wq = nc.dram_tensor(
        "wq", [128, K_CHUNKS, N_EXPERTS], xdt, kind="ExternalInput"
    ).ap()
    eye8 = nc.dram_tensor("eye8", [8, 8], f32, kind="ExternalInput").ap()
    w_out = nc.dram_tensor(
        "w_out", [TOKENS_PER_CORE, 2], f32, kind="ExternalOutput"
    ).ap()
    idx_out = nc.dram_tensor(
        "idx_out", [TOKENS_PER_CORE, 2], u32, kind="ExternalOutput"
    ).ap()

    n_blk = TOKENS_PER_CORE // 128  # 16 output blocks of 128 tokens
    w_out_v = w_out.rearrange("(n p) k -> p n k", p=128)  # [128, 16, 2]
    idx_out_v = idx_out.rearrange("(n p) k -> p n k", p=128)

    with tile.TileContext(nc) as tc:
        with (
            tc.tile_pool(name="wpool", bufs=1) as wpool,
            tc.tile_pool(name="xpool", bufs=X_BUFS) as xpool,
            tc.tile_pool(name="psum", bufs=4, space="PSUM") as psum_pool,
            tc.tile_pool(name="psumT", bufs=2, space="PSUM") as psumT_pool,
            tc.tile_pool(name="scratch", bufs=4) as scratch,
            tc.tile_pool(name="acc", bufs=1) as acc_pool,
        ):
            w_sb = wpool.tile([128, K_CHUNKS, N_EXPERTS], xdt)
            nc.sync.dma_start(w_sb[:], wq[:])
            eye_sb = wpool.tile([8, 8], f32)
            nc.sync.dma_start(eye_sb[:], eye8[:])

            w_acc = acc_pool.tile([128, n_blk, 2], f32)
            idx_acc = acc_pool.tile([128, n_blk, 2], u32)
            if variant == "dma":
                nc.vector.memset(w_acc[:], 0.0)
                nc.vector.memset(idx_acc[:], 0)

            kq = K_CHUNKS // K_SPLIT
            for tt_r in range(N_TOKEN_TILES * repeat):
                tt = tt_r % N_TOKEN_TILES
                x_parts = []
                for q in range(K_SPLIT):
                    xp = xpool.tile([128, kq, TOKEN_TILE], xdt, tag=f"x{q}")
                    if tt_r == 0 and q == 0:
                        # Halve the very first load so the PE's first matmul
                        # group starts ~3us earlier (head-fill latency).
                        h = kq // 2
                        nc.sync.dma_start(
                            xp[:, :h, :], xq[tt, :, 0:h, :]
                        )
                        nc.sync.dma_start(
                            xp[:, h:, :], xq[tt, :, h:kq, :]
                        )
                    else:
                        nc.sync.dma_start(
                            xp[:], xq[tt, :, q * kq : (q + 1) * kq, :]
                        )
                    x_parts.append(xp)

                if variant == "dma":
                    continue

                if variant in ("a", "af"):
                    # W^T stationary [128k, 8e], x moving [128k, 512t]
                    # -> scores^T [8, 512] accumulated in PSUM.
                    scT_ps = psumT_pool.tile([8, TOKEN_TILE], f32, tag="scT")
                    for k in range(K_CHUNKS):
                        nc.tensor.matmul(
                            scT_ps[:],
                            w_sb[:, k, :],
                            x_parts[k // kq][:, k % kq, :],
                            start=(k == 0),
                            stop=(k == K_CHUNKS - 1),
                        )
                    scT_sb = scratch.tile([8, TOKEN_TILE], f32, tag="scT_sb")
                    nc.scalar.copy(scT_sb[:], scT_ps[:])
                elif variant == "ac":
                    # Column-packed: 4 k-chunks run concurrently in distinct
                    # 32-column strips of the PE array (tile_position).
                    # Strip c accumulates chunks k ≡ c (mod 4) into
                    # ps4[32c:32c+8, :]; the 4 partials are summed after.
                    ps4 = psumT_pool.tile([128, TOKEN_TILE], f32, tag="scT4")
                    ng = K_CHUNKS // 4
                    for g in range(ng):
                        for c in range(4):
                            k = g * 4 + c
                            nc.tensor.matmul(
                                ps4[32 * c : 32 * c + N_EXPERTS, :],
                                w_sb[:, k, :],
                                x_parts[k // kq][:, k % kq, :],
                                start=(g == 0),
                                stop=(g == ng - 1),
                                tile_position=(0, 32 * c),
                                skip_group_check=(c != 0),
                            )
                    # Consume strips in REVERSE order: the c=3 copy's RAW dep
                    # is on the program-final matmul, and PE matmul *ends* are
                    # pc-monotone, so by the time any strip is read the whole
                    # bank is quiescent. (Reading c=0 first only waits for
                    # strip 0's last matmul — PE would still be writing strips
                    # 1-3 of the same PSUM bank: fatal same-bank W+R hazard.)
                    scT_sb = scratch.tile([8, TOKEN_TILE], f32, tag="scT_sb")
                    nc.scalar.copy(scT_sb[:], ps4[96 : 96 + N_EXPERTS, :])
                    for c in (2, 1, 0):
                        nc.vector.tensor_add(
                            scT_sb[:],
                            scT_sb[:],
                            ps4[32 * c : 32 * c + N_EXPERTS, :],
                        )

                for j in range(SUB):
                    blk = tt * SUB + j  # 128-token output block index
                    scores_ps = psum_pool.tile([128, N_EXPERTS], f32, tag="scores")
                    if a_like:
                        nc.tensor.transpose(
                            scores_ps[:],
                            scT_sb[:, j * 128 : (j + 1) * 128],
                            eye_sb[:],
                        )
                    else:
                        for k in range(K_CHUNKS):
                            nc.tensor.matmul(
                                scores_ps[:],
                                # lhsT (stationary) [128k, 128t]
                                x_parts[k // kq][
                                    :, k % kq, j * 128 : (j + 1) * 128
                                ],
                                w_sb[:, k, :],  # rhs (moving) [128k, 8e]
                                start=(k == 0),
                                stop=(k == K_CHUNKS - 1),
                            )

                    # top-8 sorted values + indices per token
                    maxv = scratch.tile([128, 8], f32, tag="maxv")
                    maxi = scratch.tile([128, 8], u32, tag="maxi")
                    nc.vector.max(maxv[:], scores_ps[:])
                    nc.vector.max_index(maxi[:], maxv[:], scores_ps[:])

                    # w1 = 1/(1+e^d), w2 = e^d/(1+e^d), d = s2-s1 <= 0
                    d = scratch.tile([128, 1], f32, tag="d")
                    nc.vector.tensor_sub(d[:], maxv[:, 1:2], maxv[:, 0:1])
                    e = scratch.tile([128, 1], f32, tag="e")
                    nc.scalar.activation(
                        e[:], d[:], mybir.ActivationFunctionType.Exp
                    )
                    den = scratch.tile([128, 1], f32, tag="den")
                    nc.scalar.activation(
                        den[:], e[:], mybir.ActivationFunctionType.Copy, bias=1.0
                    )
                    w1 = w_acc[:, blk, 0:1]
                    w2 = w_acc[:, blk, 1:2]
                    nc.vector.reciprocal(w1, den[:])
                    nc.vector.tensor_mul(w2, e[:], w1)
                    nc.vector.tensor_copy(idx_acc[:, blk, :], maxi[:, 0:2])

                if variant != "dma" and tt_r >= N_TOKEN_TILES * (repeat - 1):
                    # Final pass over the data: flush this tile's outputs now
                    # so only the last tile's 4KB DMAs sit in the tail.
                    b0, b1 = tt * SUB, (tt + 1) * SUB
                    nc.sync.dma_start(
                        w_out_v[:, b0:b1, :], w_acc[:, b0:b1, :]
                    )
                    nc.sync.dma_start(
                        idx_out_v[:, b0:b1, :], idx_acc[:, b0:b1, :]
                    )

            if variant == "dma":
                nc.sync.dma_start(w_out_v[:], w_acc[:])
                nc.sync.dma_start(idx_out_v[:], idx_acc[:])

    _split_multiwait(nc)
    return nc


_CACHED_NC = {}


def make_in_maps(x: np.ndarray, W_router: np.ndarray):
    x_flat = np.ascontiguousarray(x, dtype=np.float32).reshape(-1, HIDDEN)
    wT_np = np.ascontiguousarray(W_router.T, dtype=np.float32)
    in_maps = []
    for c in range(N_CORES):
        sl = x_flat[c * TOKENS_PER_CORE : (c + 1) * TOKENS_PER_CORE]
        # [2048, 4096] -> [n, ti, k, p] -> [n, p, k, ti] contiguous
        xq = np.ascontiguousarray(
            sl.reshape(N_TOKEN_TILES, TOKEN_TILE, K_CHUNKS, 128).transpose(
                0, 3, 2, 1
            )
        )
        wq = np.ascontiguousarray(
            wT_np.reshape(K_CHUNKS, 128, N_EXPERTS).transpose(1, 0, 2)
        )
        in_maps.append(
            {"xq": xq, "wq": wq, "eye8": np.eye(8, dtype=np.float32)}
        )
    return in_maps


def kernel(x: np.ndarray, W_router: np.ndarray):
    global LAST_EXEC_NS, _CACHED_NC

    in_maps = make_in_maps(x, W_router)

    variant = os.environ.get("TRN_KERNEL_VARIANT", DEFAULT_VARIANT)
    if variant not in _CACHED_NC:
        _CACHED_NC[variant] = build_kernel(variant=variant)
    nc = _CACHED_NC[variant]

    trace = bool(int(os.environ.get("TRN_KERNEL_TRACE", "0")))
    res = bass_utils.run_bass_kernel_spmd(
        nc, in_maps, core_ids=list(range(N_CORES)), trace=trace
    )
    LAST_EXEC_NS = res.exec_time_ns

    weights = np.concatenate([r["w_out"] for r in res.results], axis=0)
    indices = np.concatenate(
        [r["idx_out"].astype(np.int32) for r in res.results], axis=0
    )
    return weights, indices
